# revision 3
# baseline (speedup 1.0000x reference)
"""Trainium2 Bass kernel v3 for nn_Decoder (ragged LSTM decoder), 8-core SPMD.

v2 + phase-B latency hiding and ACT-table hygiene:
  - Phase B splits the 16-row batch into 2 independent streams of 8; their
    serial pointwise chains hide behind each other's matmul bursts.
  - All gates via tanh only (sigmoid(x)=0.5(1+tanh(x/2)) with the 0.5/2
    factors folded into host-side weight scaling; state kept as C~=2c,
    H~=2h) -> single ACT op per stream-step, one ACT table set for the
    whole kernel (exp_and_others), no per-step table thrash.
  - Pointwise chain uses scalar_tensor_tensor (1 op per fused (x+1)*y).
  - Phase C: log(sumexp) batched into one Ln at the end (one table swap).
Column order everywhere: col = t*16 + b_local (t-major).
"""
import sys
sys.path.insert(0, "/opt/trn_rl_repo")

import numpy as np
import ml_dtypes

B, T, H = 128, 512, 384
D_ENC, HID = 768, 768
POS_SIZE, POS_DIM, LABEL = 64, 128, 128
APP_ID = 3
NCORES = 8
BC = B // NCORES          # 16
COLS = T * BC             # 8192
NCH = 16                  # phase A chunks of 512 cols
NS = 2                    # phase B batch streams
SB = BC // NS             # 8 batch rows per stream
# gate j-tile permutation: [i(0:3), f(3:6), o(9:12), g(6:9)]
GPERM = [0, 1, 2, 3, 4, 5, 9, 10, 11, 6, 7, 8]

_COMPILED = None


def _build(reps=1, phases='abc'):
    import concourse.bass as bass
    import concourse.mybir as mybir
    import concourse.tile as tile
    from concourse import bacc
    from contextlib import ExitStack

    f32 = mybir.dt.float32
    bf16 = mybir.dt.bfloat16
    AF = mybir.ActivationFunctionType
    ALU = mybir.AluOpType

    nc = bacc.Bacc(None, target_bir_lowering=False, debug=False,
                   num_devices=NCORES)

    def param(name, shape, dt=f32):
        return nc.declare_dram_parameter(name, list(shape), dt, isOutput=False)

    xT = param("xT", [7, 128, COLS], bf16)
    encT = param("encT", [128, T // 8, 6, 128], bf16)
    combWT = param("combWT", [7, 128, HID], bf16)
    wihT = param("wihT", [6, 128, 4 * H], bf16)
    whhT = param("whhT", [3, 128, 4 * H], bf16)
    outWhT = param("outWhT", [3, 128, LABEL], bf16)
    outWeT = param("outWeT", [6, 128, LABEL], bf16)
    id128 = param("id128", [128, 128], bf16)
    combb = param("combb", [6, 128])
    biassum = param("biassum", [12, 128])

    out = nc.declare_dram_parameter("out", [BC, T, LABEL], f32, isOutput=True)

    gin_d = nc.dram_tensor("gin_d", [128, 32, 12, 16, BC], bf16)

    with tile.TileContext(nc) as tc, ExitStack() as top:
        singles = top.enter_context(tc.tile_pool(name="singles", bufs=1))

        combWT_sb = singles.tile([128, 7, HID], bf16)
        wihT_sb = singles.tile([128, 6, 4 * H], bf16)
        whhT_sb = singles.tile([128, 3, 4 * H], bf16)
        outWhT_sb = singles.tile([128, 3, LABEL], bf16)
        outWeT_sb = singles.tile([128, 6, LABEL], bf16)
        id_sb = singles.tile([128, 128], bf16)
        combb_sb = singles.tile([128, 6], f32)
        biassum_sb = singles.tile([128, 12], f32)
        h2_all = singles.tile([128, 3, COLS], bf16)
        xm_all = singles.tile([128, COLS // 128, 128], f32)
        ssum_all = singles.tile([128, COLS // 128], f32)

        nc.sync.dma_start(out=combWT_sb, in_=combWT.ap().rearrange("k p m -> p k m"))
        nc.sync.dma_start(out=wihT_sb, in_=wihT.ap().rearrange("k p m -> p k m"))
        nc.sync.dma_start(out=whhT_sb, in_=whhT.ap().rearrange("k p m -> p k m"))
        nc.sync.dma_start(out=outWhT_sb, in_=outWhT.ap().rearrange("k p m -> p k m"))
        nc.sync.dma_start(out=outWeT_sb, in_=outWeT.ap().rearrange("k p m -> p k m"))
        nc.sync.dma_start(out=id_sb, in_=id128.ap())
        nc.sync.dma_start(out=combb_sb, in_=combb.ap().rearrange("m p -> p m"))
        nc.sync.dma_start(out=biassum_sb, in_=biassum.ap().rearrange("m p -> p m"))

        for _rep in range(reps):
            # ================= Phase A =================
            with ExitStack() as pa:
              if 'a' in phases:
                with nc.named_scope("phaseA"):
                    xt_pool = pa.enter_context(tc.tile_pool(name="xt", bufs=3))
                    z_pool = pa.enter_context(tc.tile_pool(name="zt", bufs=2))
                    g_pool = pa.enter_context(tc.tile_pool(name="ginw", bufs=3))
                    psA = pa.enter_context(tc.tile_pool(name="psA", bufs=2, space="PSUM"))
                    psB = pa.enter_context(tc.tile_pool(name="psB", bufs=2, space="PSUM"))

                    for ch in range(NCH):
                        c0 = ch * 512
                        xt = xt_pool.tile([128, 7, 512], bf16)
                        nc.sync.dma_start(
                            out=xt,
                            in_=xT.ap()[:, :, c0:c0 + 512].rearrange("k p c -> p k c"))
                        zT = z_pool.tile([128, 6, 512], bf16)
                        for m in range(6):
                            ps = psA.tile([128, 512], f32)
                            for k in range(7):
                                nc.tensor.matmul(
                                    ps, combWT_sb[:, k, m * 128:(m + 1) * 128],
                                    xt[:, k, :], start=(k == 0), stop=(k == 6))
                            nc.scalar.activation(zT[:, m, :], ps, AF.Tanh,
                                                 bias=combb_sb[:, m:m + 1])
                        if ch == 0:
                            nc.vector.memset(zT[:, :, 0:BC], 0.0)
                        for j in range(12):
                            ps = psB.tile([128, 512], f32)
                            for k in range(6):
                                nc.tensor.matmul(
                                    ps, wihT_sb[:, k, j * 128:(j + 1) * 128],
                                    zT[:, k, :], start=(k == 0), stop=(k == 5))
                            gsb = g_pool.tile([128, 512], bf16)
                            nc.scalar.activation(gsb, ps, AF.Identity,
                                                 bias=biassum_sb[:, j:j + 1])
                            nc.sync.dma_start(
                                out=gin_d.ap()[:, 2 * ch:2 * ch + 2, j, :, :],
                                in_=gsb[:, :].rearrange("p (x s b) -> p x s b",
                                                        x=2, s=16, b=BC))

            # ================= Phase B: scan =================
            with ExitStack() as pb:
              if 'b' in phases:
                with nc.named_scope("phaseB"):
                    gpool = pb.enter_context(tc.tile_pool(name="ginr", bufs=3))
                    cpool = pb.enter_context(tc.tile_pool(name="cp", bufs=2))
                    gapool = pb.enter_context(tc.tile_pool(name="ga", bufs=2))
                    tmp = pb.enter_context(tc.tile_pool(name="stmp", bufs=2))
                    psS = pb.enter_context(tc.tile_pool(name="psS", bufs=1, space="PSUM"))

                    cT = cpool.tile([128, 3 * BC], f32, tag="c")
                    nc.vector.memset(cT, 0.0)
                    # gate regions in separate PSUM banks so each gate's tanh
                    # can read its bank while later gates' matmuls still write
                    # theirs; matmul order g,i,f,o matches consumer order.
                    GJ = {"g": [9, 10, 11], "i": [0, 1, 2],
                          "f": [3, 4, 5], "o": [6, 7, 8]}
                    GORD = ["g", "i", "f", "o"]
                    for blk in range(32):
                        gch = gpool.tile([128, 12, 16, BC], bf16)
                        nc.sync.dma_start(out=gch, in_=gin_d.ap()[:, blk, :, :, :])
                        for s in range(16):
                            t = blk * 16 + s
                            psg = {}
                            for gn in GORD:
                                p = psS.tile([128, 3 * BC], f32, tag=f"ps_{gn}")
                                j0 = GJ[gn][0]
                                nc.tensor.matmul(p, id_sb,
                                                 gch[:, j0:j0 + 3, s, :],
                                                 start=True, stop=(t == 0))
                                psg[gn] = p
                            if t > 0:
                                hp = h2_all[:, :, (t - 1) * BC:t * BC]
                                for gn in GORD:
                                    for jj, j in enumerate(GJ[gn]):
                                        for k in range(3):
                                            nc.tensor.matmul(
                                                psg[gn][:, jj * BC:(jj + 1) * BC],
                                                whhT_sb[:, k, j * 128:(j + 1) * 128],
                                                hp[:, k, :],
                                                start=False, stop=(k == 2))
                            gact = gapool.tile([128, 12 * BC], f32)
                            for gi, gn in enumerate(GORD):
                                nc.scalar.activation(
                                    gact[:, gi * 3 * BC:(gi + 1) * 3 * BC],
                                    psg[gn], AF.Tanh)
                            tg = gact[:, 0:3 * BC]
                            yi = gact[:, 3 * BC:6 * BC]
                            yf = gact[:, 6 * BC:9 * BC]
                            yo = gact[:, 9 * BC:12 * BC]
                            t2 = tmp.tile([128, 3 * BC], f32, tag="t2")
                            nc.vector.scalar_tensor_tensor(
                                t2, yi, 1.0, tg, ALU.add, ALU.mult)
                            t1 = tmp.tile([128, 3 * BC], f32, tag="t1")
                            nc.vector.scalar_tensor_tensor(
                                t1, yf, 1.0, cT, ALU.add, ALU.mult)
                            cN = cpool.tile([128, 3 * BC], f32, tag="c")
                            nc.vector.scalar_tensor_tensor(
                                cN, t1, 0.5, t2, ALU.mult, ALU.add)
                            cT = cN
                            tc2 = tmp.tile([128, 3 * BC], f32, tag="tc")
                            nc.scalar.activation(tc2, cN, AF.Tanh, scale=0.5)
                            nc.vector.scalar_tensor_tensor(
                                h2_all[:, :, t * BC:(t + 1) * BC],
                                yo.rearrange("p (k b) -> p k b", k=3, b=BC),
                                1.0,
                                tc2[:, :].rearrange("p (k b) -> p k b", k=3, b=BC),
                                ALU.add, ALU.mult)

            # ================= Phase C =================
            with ExitStack() as pc:
              if 'c' in phases:
                with nc.named_scope("phaseC"):
                    encr = pc.enter_context(tc.tile_pool(name="encr", bufs=3))
                    smp = pc.enter_context(tc.tile_pool(name="smp", bufs=4))
                    smc = pc.enter_context(tc.tile_pool(name="smc", bufs=4))
                    psc = pc.enter_context(tc.tile_pool(name="psC", bufs=2, space="PSUM"))

                    for ch in range(COLS // 128):
                        enct = encr.tile([128, 6, 128], bf16)
                        nc.sync.dma_start(out=enct, in_=encT.ap()[:, ch, :, :])
                        ps = psc.tile([128, LABEL], f32)
                        for k in range(3):
                            nc.tensor.matmul(ps, h2_all[:, k, ch * 128:(ch + 1) * 128],
                                             outWhT_sb[:, k, :],
                                             start=(k == 0), stop=False)
                        for k in range(6):
                            nc.tensor.matmul(ps, enct[:, k, :], outWeT_sb[:, k, :],
                                             start=False, stop=(k == 5))
                        lg = smp.tile([128, LABEL], f32, tag="lg")
                        nc.vector.tensor_copy(lg, ps)
                        if ch == 0:
                            nc.vector.memset(lg[0:BC, APP_ID:APP_ID + 1], -1e10)
                        mx = smc.tile([128, 1], f32, tag="mx")
                        nc.vector.tensor_reduce(mx, lg, mybir.AxisListType.X, ALU.max)
                        nc.vector.tensor_scalar(xm_all[:, ch, :], lg, mx, None,
                                                ALU.subtract)
                        et = smp.tile([128, LABEL], f32, tag="et")
                        nc.scalar.activation(et, xm_all[:, ch, :], AF.Exp,
                                             accum_out=ssum_all[:, ch:ch + 1])
                    lns_all = singles.tile([128, COLS // 128], f32)
                    nc.scalar.activation(lns_all, ssum_all, AF.Ln)
                    for ch in range(COLS // 128):
                        res = smp.tile([128, LABEL], f32, tag="res")
                        nc.vector.tensor_scalar(res, xm_all[:, ch, :],
                                                lns_all[:, ch:ch + 1], None,
                                                ALU.subtract)
                        nc.sync.dma_start(
                            out=out.ap().rearrange("b t l -> t b l")[ch * 8:(ch + 1) * 8, :, :],
                            in_=res)

    nc.compile()
    return nc


def _host_prep(encoder_out, pos_embed_w, W_ih, W_hh, b_ih, b_hh,
               combine_W, combine_b, out_W, word_start, pos_ids):
    bf = ml_dtypes.bfloat16
    enc = np.asarray(encoder_out, dtype=np.float32)        # [B, T, 768]
    ws = np.asarray(word_start)                            # [T, B]
    pid = np.asarray(pos_ids)                              # [T, B]
    posw = np.asarray(pos_embed_w, np.float32)

    # ragged word average (host): word[t,b,:] = mean(enc[b, s:t, :]) or 0
    enc_t = enc.transpose(1, 0, 2).astype(np.float64)      # [T, B, 768]
    csum = np.concatenate([np.zeros((1, B, D_ENC)), np.cumsum(enc_t, axis=0)],
                          axis=0)                          # [T+1, B, 768]
    tgrid = np.arange(T)[:, None]
    valid = ws >= 0
    s = np.clip(ws, 0, None)
    ln = np.maximum(tgrid - s, 1).astype(np.float64)
    bidx = np.arange(B)
    word = (csum[tgrid, bidx[None, :], :] - csum[s, bidx[None, :], :]) \
        / ln[:, :, None]
    word = np.where(valid[:, :, None], word, 0.0).astype(np.float32)  # [T,B,768]
    pos_e = posw[pid]                                      # [T, B, 128]
    x = np.concatenate([pos_e, word], axis=2)              # [T, B, 896]

    # gate j-tile permutation [i, f, o, g] + all-tanh scale folding:
    #   i,f,o pre-activations halved (gin and W_hh rows); h stored as 2h
    #   (W_hh and out_W h-columns halved).
    Wih = np.asarray(W_ih, np.float32).reshape(12, 128, HID)[GPERM]
    Whh = np.asarray(W_hh, np.float32).reshape(12, 128, H)[GPERM]
    bsum = (np.asarray(b_ih, np.float32) + np.asarray(b_hh, np.float32)
            ).reshape(12, 128)[GPERM].copy()
    Wih[0:9] *= 0.5
    bsum[0:9] *= 0.5
    Whh = Whh * 0.5                  # h~ = 2h compensation
    Whh[0:9] *= 0.5                  # i,f,o preact halving
    Wih = Wih.reshape(4 * H, HID)
    Whh = Whh.reshape(4 * H, H)
    outW = np.asarray(out_W, np.float32)
    outWh = outW[:, :H] * 0.5        # h~ = 2h compensation

    shared = dict(
        combWT=np.ascontiguousarray(
            np.asarray(combine_W, np.float32).T).reshape(7, 128, HID).astype(bf),
        wihT=np.ascontiguousarray(Wih.T).reshape(6, 128, 4 * H).astype(bf),
        whhT=np.ascontiguousarray(Whh.T).reshape(3, 128, 4 * H).astype(bf),
        outWhT=np.ascontiguousarray(outWh.T).reshape(3, 128, LABEL).astype(bf),
        outWeT=np.ascontiguousarray(outW[:, H:].T).reshape(6, 128, LABEL).astype(bf),
        id128=np.eye(128, dtype=np.float32).astype(bf),
        combb=np.asarray(combine_b, np.float32).reshape(6, 128),
        biassum=np.ascontiguousarray(bsum),
    )
    in_maps = []
    for c in range(NCORES):
        bs = slice(c * BC, (c + 1) * BC)
        m = dict(shared)
        xc = x[:, bs, :].transpose(2, 0, 1).reshape(896, COLS)
        m["xT"] = np.ascontiguousarray(xc).astype(bf).reshape(7, 128, COLS)
        ec = enc[bs].transpose(2, 1, 0).reshape(768, COLS)   # [d, t*16+b]
        ec = ec.reshape(6, 128, 64, 128).transpose(1, 2, 0, 3)
        m["encT"] = np.ascontiguousarray(ec).astype(bf)
        in_maps.append(m)
    return in_maps


def _get_compiled():
    global _COMPILED
    if _COMPILED is None:
        import os
        reps = int(os.environ.get("BK_REPS", "1"))
        phases = os.environ.get("BK_PHASES", "abc")
        _COMPILED = _build(reps=reps, phases=phases)
    return _COMPILED


def kernel(**inputs):
    from concourse.bass_utils import run_bass_kernel_spmd
    nc = _get_compiled()
    in_maps = _host_prep(**inputs)
    res = run_bass_kernel_spmd(nc, in_maps, list(range(NCORES)))
    outs = [res.results[c]["out"] for c in range(NCORES)]
    full = np.concatenate(outs, axis=0)           # [B, T, LABEL]
    return full.reshape(B * T, LABEL).astype(np.float32)
